# Initial kernel scaffold
#
"""Trainium2 Bass kernel for nn_CrossOutLayer_2 (dense pairwise MLP).

o[b,n,m] = sum_e W2[e]*gelu(hx[b,n,e] + hy[b,m,e] + b1[e]) + b2
  hx = x0 @ W1[:D] + x @ W1[D:2D],  hy = y @ W1[2D:]

Instead of evaluating gelu on all B*N1*N2*D elements (ACT-bound, ~66us),
approximate gelu with a separable harmonic expansion

  gelu(s) ~ g0 + 0.5 s + g2 s^2 + sum_{k=1..K} a_k cos(k w0 s)   (K=4: 1.3e-3)

Each term is separable in s = a + b (a = hx+b1 per n, b = hy per m) via the
angle-addition formula, so the whole pairwise map becomes 18 rank-128 fp16
matmuls on the PE:  o = sum_r F_r(a).T @ G_r(b):
  G basis {1, b, b^2, c1^j, s1*c1^(j-1)}: c1/s1 by ACT Sin (args <= 3.4 rad,
    inside the Sin spline's accurate range), monomials one fp16 DVE multiply
    each.
  F side: ACT Sin only at half-angle (args <= 2.9 rad), then fp16 Chebyshev
    recurrences on DVE for harmonics 2..K; per-partition scale/bias APs fold
    b1, phases, and w2; tensor_scalar applies the w2*coeff weights.

Sharded over (b, n1): each core owns 128 n-rows and all m.  Inputs packed
fp16 into one DMA; output written [n, m]-layout directly.
"""

import sys

sys.path.insert(0, "/opt/trn_rl_repo")

import numpy as np

B, N1, N2, D = 2, 512, 512, 128
NCORES = 8
ROWS = B * N1 // NCORES  # 128 n-rows per core
PKW = 128 * 5 + 512      # packed input width (x0T, xT, yT, Wa, Wb, Wc)
NCV = 25                 # const-vector columns

# fit constants (gelu ~ g0 + 0.5 s + g2 s^2 + sum_{k<=4} a_k cos(k w0 s))
FIT_G0 = 0.7486143130301098
FIT_W0 = 0.6532571942412266
FIT_A = (-0.5604016227440258, -0.14082111584080594,
         -0.03847842242402837, -0.007632327159040924)
FIT_G2 = 0.05209814114155775

_cache = {}


def _build(repeat=1, nmm=99, gchain=True, frec=True, weights=True,
           gacts=True, facts=True, dma_in=True, dma_out=True, pregemm=True,
           mmcombine=True, pipe_mode=True, unroll=2):
    key = ("nc", repeat, nmm, gchain, frec, weights, gacts, facts, dma_in,
           dma_out, pregemm, mmcombine, pipe_mode, unroll)
    if key in _cache:
        return _cache[key]
    import concourse.bacc as bacc
    import concourse.mybir as mybir
    import concourse.tile as tile

    f32 = mybir.dt.float32
    f16 = mybir.dt.float16
    SIN = mybir.ActivationFunctionType.Sin
    SQUARE = mybir.ActivationFunctionType.Square
    IDENT = mybir.ActivationFunctionType.Identity
    MULT = mybir.AluOpType.mult
    ADD = mybir.AluOpType.add
    w0 = FIT_W0

    nc = bacc.Bacc("TRN2", target_bir_lowering=False, debug=False)
    pk = nc.dram_tensor("pk", [D, PKW], f16, kind="ExternalInput")
    cv = nc.dram_tensor("cv", [D, NCV], f32, kind="ExternalInput")
    outT = nc.dram_tensor("outT", [D, N2], f32, kind="ExternalOutput")

    with tile.TileContext(nc) as tc:
        with (
            tc.tile_pool(name="const", bufs=1) as cpool,
            tc.tile_pool(name="work", bufs=2) as wpool,
            tc.tile_pool(name="psum", bufs=2, space="PSUM") as pspool,
        ):
            # ---- persistent constants (outside repeat loop) ----
            cv_sb = cpool.tile([D, NCV], f32, name="cv_sb", tag="cv_sb")
            nc.sync.dma_start(cv_sb[:], cv[:])
            ones_sb = cpool.tile([D, N2], f16, name="ones_sb", tag="ones_sb")
            nc.vector.memset(ones_sb[:], 1.0)
            fgq = cpool.tile([D, ROWS], f16, name="fgq", tag="fgq")
            nc.vector.tensor_scalar_mul(fgq[:], ones_sb[:, :ROWS], cv_sb[:, 10:11])

            def c(i):
                return cv_sb[:, i:i + 1]

            def load_stage(pipe, iv):
                pk_t = pipe.intermediate_tile([D, PKW], f16, name="pk_t")
                if dma_in:
                    nc.sync.dma_start(pk_t[:], pk[:])
                elif pregemm:
                    nc.vector.memset(pk_t[:], 0.01)
                return pk_t

            def compute_stage(pipe, iv, pk_sb):
                o_t = pipe.intermediate_tile([D, N2], f32, name="o_t")
                compute_body(pk_sb, o_t)
                return o_t

            def store_stage(pipe, iv, o_t):
                if dma_out:
                    nc.sync.dma_start(outT[:], o_t[:])

            def compute_body(pk_sb, o_sb):
                x0T = pk_sb[:, 0:128]
                xT = pk_sb[:, 128:256]
                yT = pk_sb[:, 256:768]
                Wa = pk_sb[:, 768:896]
                Wb = pk_sb[:, 896:1024]
                Wc = pk_sb[:, 1024:1152]

                hy_ps = pspool.tile([D, N2], f32, name="hy_ps", tag="hy")
                hx_ps = pspool.tile([D, N2], f32, name="hx_ps", tag="hx")
                if pregemm:
                    nc.tensor.matmul(hy_ps[:], Wc, yT, start=True, stop=True)
                    nc.tensor.matmul(hx_ps[:, :ROWS], Wa, x0T, start=True, stop=False)
                    nc.tensor.matmul(hx_ps[:, :ROWS], Wb, xT, start=False, stop=True)
                else:
                    nc.vector.memset(hy_ps[:], 0.1)
                    nc.vector.memset(hx_ps[:, :ROWS], 0.1)
                hx = hx_ps[:, :ROWS]

                def gtile(name):
                    return cpool.tile([D, N2], f16, name=name, tag=name)

                def ftile(name):
                    return cpool.tile([D, ROWS], f16, name=name, tag=name)

                # ---- ACT: G base first (gates the longest DVE chain) ----
                c1 = gtile("c1")
                s1 = gtile("s1")
                g1 = gtile("g1")
                gq = gtile("gq")
                if gacts:
                    nc.scalar.activation(c1[:], hy_ps[:], SIN, bias=c(7), scale=w0)
                    nc.scalar.activation(s1[:], hy_ps[:], SIN, bias=c(8), scale=w0)
                shA = ftile("shA")
                chA = ftile("chA")
                sqraw = ftile("sqraw")
                fm1 = ftile("fm1")
                fm5 = ftile("fm5")
                if facts:
                    nc.scalar.activation(shA[:], hx, SIN, bias=c(5), scale=w0 / 2)
                    nc.scalar.activation(chA[:], hx, SIN, bias=c(6), scale=w0 / 2)
                if gacts:
                    nc.scalar.activation(g1[:], hy_ps[:], IDENT, bias=c(8), scale=1.0)
                    nc.scalar.activation(gq[:], hy_ps[:], SQUARE, bias=c(8), scale=1.0)
                if facts:
                    nc.scalar.activation(sqraw[:], hx, SQUARE, bias=c(0), scale=1.0)
                    nc.scalar.activation(fm1[:], hx, IDENT, bias=c(2), scale=c(1))
                    nc.scalar.activation(fm5[:], hx, IDENT, bias=c(4), scale=c(3))
                if not gacts:
                    nc.vector.memset(c1[:], 0.3)
                    nc.vector.memset(s1[:], 0.3)
                    nc.vector.memset(g1[:], 0.3)
                    nc.vector.memset(gq[:], 0.3)
                if not facts:
                    nc.vector.memset(shA[:], 0.3)
                    nc.vector.memset(chA[:], 0.3)
                    nc.vector.memset(sqraw[:], 0.3)
                    nc.vector.memset(fm1[:], 0.3)
                    nc.vector.memset(fm5[:], 0.3)

                # ---- DVE: G monomial chain ----
                if gchain:
                    u2 = gtile("u2")
                    nc.vector.tensor_mul(u2[:], c1[:], c1[:])
                    v2 = gtile("v2")
                    nc.vector.tensor_mul(v2[:], s1[:], c1[:])
                    u3 = gtile("u3")
                    nc.vector.tensor_mul(u3[:], u2[:], c1[:])
                    v3 = gtile("v3")
                    nc.vector.tensor_mul(v3[:], v2[:], c1[:])
                    u4 = gtile("u4")
                    nc.vector.tensor_mul(u4[:], u3[:], c1[:])
                    v4 = gtile("v4")
                    nc.vector.tensor_mul(v4[:], v3[:], c1[:])
                else:
                    u2 = v2 = u3 = v3 = u4 = v4 = c1

                # ---- DVE: F-side fp16 Chebyshev recurrences ----
                if not frec:
                    c1r = ss = c2r = s2r = c3r = s3r = c4r = s4r = shA
                if frec:
                    sh2A = ftile("sh2A")
                    nc.vector.tensor_mul(sh2A[:], shA[:], shA[:])
                    c1r = ftile("c1r")  # cos(w0 a)
                    nc.vector.tensor_scalar(c1r[:], sh2A[:], -2.0, 1.0, MULT, ADD)
                    ss = ftile("ss")    # sin(w0 a)/2
                    nc.vector.tensor_mul(ss[:], shA[:], chA[:])
                    c1dd = ftile("c1dd")
                    nc.vector.tensor_scalar_mul(c1dd[:], c1r[:], 2.0)
                    c2t = ftile("c2t")
                    nc.vector.tensor_mul(c2t[:], c1dd[:], c1r[:])
                    c2r = ftile("c2r")
                    nc.vector.tensor_scalar_add(c2r[:], c2t[:], -1.0)
                    s2r = ftile("s2r")  # sin(2 w0 a)/2
                    nc.vector.tensor_mul(s2r[:], c1dd[:], ss[:])
                    c3t = ftile("c3t")
                    nc.vector.tensor_mul(c3t[:], c1dd[:], c2r[:])
                    c3r = ftile("c3r")
                    nc.vector.tensor_sub(c3r[:], c3t[:], c1r[:])
                    s3t = ftile("s3t")
                    nc.vector.tensor_mul(s3t[:], c1dd[:], s2r[:])
                    s3r = ftile("s3r")
                    nc.vector.tensor_sub(s3r[:], s3t[:], ss[:])
                    c4t = ftile("c4t")
                    nc.vector.tensor_mul(c4t[:], c1dd[:], c3r[:])
                    c4r = ftile("c4r")
                    nc.vector.tensor_sub(c4r[:], c4t[:], c2r[:])
                    s4t = ftile("s4t")
                    nc.vector.tensor_mul(s4t[:], c1dd[:], s3r[:])
                    s4r = ftile("s4r")
                    nc.vector.tensor_sub(s4r[:], s4t[:], s2r[:])

                # ---- DVE: per-partition weights ----
                def ts(name, raw, ci):
                    if not weights:
                        return raw
                    t = ftile(name)
                    nc.vector.tensor_scalar_mul(t[:], raw[:], c(ci))
                    return t

                def stt(name, raw, ci, addend):
                    t = ftile(name)
                    nc.vector.scalar_tensor_tensor(
                        t[:], raw[:], c(ci), addend[:], MULT, ADD
                    )
                    return t

                o_ps = pspool.tile([D, N2], f32, name="o_ps", tag="o_ps")
                if mmcombine and weights and frec:
                    # fold all terms sharing a G feature into one stationary
                    t1 = stt("t1", sqraw, 10, fm1)
                    t2 = stt("t2", c2r, 11, t1)
                    st_ones = stt("st_ones", c4r, 12, t2)
                    st_c1 = stt("st_c1", c3r, 14, ts("c1w", c1r, 13))
                    st_s1 = stt("st_s1", s3r, 20, ts("s1w", ss, 19))
                    st_u2 = stt("st_u2", c4r, 16, ts("c2wb", c2r, 15))
                    st_v2 = stt("st_v2", s4r, 22, ts("s2w", s2r, 21))
                    mms = [
                        (fm5, g1),
                        (fgq, gq),
                        (st_c1, c1),
                        (st_s1, s1),
                        (st_u2, u2),
                        (st_v2, v2),
                        (ts("c3wb", c3r, 17), u3),
                        (ts("s3wb", s3r, 23), v3),
                        (ts("c4wc", c4r, 18), u4),
                        (ts("s4wb", s4r, 24), v4),
                        (st_ones, ones_sb),
                    ]
                else:
                    sqw = ts("sqw", sqraw, 10)
                    c2wa = ts("c2wa", c2r, 11)
                    c4wa = ts("c4wa", c4r, 12)
                    c1w = ts("c1w", c1r, 13)
                    c3wa = ts("c3wa", c3r, 14)
                    c2wb = ts("c2wb", c2r, 15)
                    c4wb = ts("c4wb", c4r, 16)
                    c3wb = ts("c3wb", c3r, 17)
                    c4wc = ts("c4wc", c4r, 18)
                    s1w = ts("s1w", ss, 19)
                    s3wa = ts("s3wa", s3r, 20)
                    s2w = ts("s2w", s2r, 21)
                    s4wa = ts("s4wa", s4r, 22)
                    s3wb = ts("s3wb", s3r, 23)
                    s4wb = ts("s4wb", s4r, 24)
                    mms = [
                        (fm1, ones_sb),
                        (fm5, g1),
                        (c1w, c1),
                        (s1w, s1),
                        (sqw, ones_sb),
                        (fgq, gq),
                        (c2wa, ones_sb),
                        (c2wb, u2),
                        (s2w, v2),
                        (c3wa, c1),
                        (c3wb, u3),
                        (s3wa, s1),
                        (s3wb, v3),
                        (c4wa, ones_sb),
                        (c4wb, u2),
                        (c4wc, u4),
                        (s4wa, v2),
                        (s4wb, v4),
                    ]
                mms = mms[:nmm]
                for i, (F, G) in enumerate(mms):
                    nc.tensor.matmul(
                        o_ps[:], F[:], G[:],
                        start=(i == 0), stop=(i == len(mms) - 1),
                    )

                # ---- evac + b2 ----
                nc.scalar.activation(o_sb[:], o_ps[:], IDENT, bias=c(9), scale=1.0)

            def body():
                pk_sb = wpool.tile([D, PKW], f16, name="pk_sb", tag="pk_sb")
                if dma_in:
                    nc.sync.dma_start(pk_sb[:], pk[:])
                elif pregemm:
                    nc.vector.memset(pk_sb[:], 0.01)
                o_sb = cpool.tile([D, N2], f32, name="o_sb", tag="o_sb")
                compute_body(pk_sb, o_sb)
                if dma_out:
                    nc.sync.dma_start(outT[:], o_sb[:])

            if repeat == 1:
                body()
            elif pipe_mode:
                tc.For_i_pipelined(
                    [load_stage, compute_stage, store_stage], 0, repeat,
                    unroll=unroll, hint_engines=(mybir.EngineType.PE,),
                )
            else:
                with tc.For_i(0, repeat, 1, hint_engines=(mybir.EngineType.PE,)):
                    body()

    nc.compile()
    _cache[key] = nc
    return nc


def _prep_in_maps(x0, x, y, W1, b1, W2, b2):
    x0 = np.asarray(x0, np.float32)
    x = np.asarray(x, np.float32)
    y = np.asarray(y, np.float32)
    W1 = np.asarray(W1, np.float32)
    b1 = np.asarray(b1, np.float32)
    W2 = np.asarray(W2, np.float32)
    b2 = np.asarray(b2, np.float32)

    w2 = W2[:, 0]
    g0, w0, a, g2 = FIT_G0, FIT_W0, FIT_A, FIT_G2

    cvm = np.zeros((D, NCV), np.float32)
    cvm[:, 0] = b1
    cvm[:, 1] = 0.5 * w2
    cvm[:, 2] = w2 * (g0 + 0.5 * b1)
    cvm[:, 3] = 2.0 * g2 * w2
    cvm[:, 4] = w2 * (0.5 + 2.0 * g2 * b1)
    cvm[:, 5] = (w0 / 2) * b1
    cvm[:, 6] = (w0 / 2) * b1 + np.pi / 2
    cvm[:, 7] = np.pi / 2
    cvm[:, 8] = 0.0
    cvm[:, 9] = b2[0]
    cvm[:, 10] = w2 * g2
    cvm[:, 11] = -w2 * a[1]
    cvm[:, 12] = w2 * a[3]
    cvm[:, 13] = w2 * a[0]
    cvm[:, 14] = -3.0 * w2 * a[2]
    cvm[:, 15] = 2.0 * w2 * a[1]
    cvm[:, 16] = -8.0 * w2 * a[3]
    cvm[:, 17] = 4.0 * w2 * a[2]
    cvm[:, 18] = 8.0 * w2 * a[3]
    cvm[:, 19] = -2.0 * w2 * a[0]
    cvm[:, 20] = 2.0 * w2 * a[2]
    cvm[:, 21] = -4.0 * w2 * a[1]
    cvm[:, 22] = 8.0 * w2 * a[3]
    cvm[:, 23] = -8.0 * w2 * a[2]
    cvm[:, 24] = -16.0 * w2 * a[3]
    cvm = np.ascontiguousarray(cvm)

    Wa16 = W1[:D].astype(np.float16)
    Wb16 = W1[D:2 * D].astype(np.float16)
    Wc16 = W1[2 * D:].astype(np.float16)

    in_maps = []
    for ci in range(NCORES):
        b = ci // (N1 // ROWS)
        n0 = (ci % (N1 // ROWS)) * ROWS
        pkm = np.empty((D, PKW), np.float16)
        pkm[:, 0:128] = x0[b, n0:n0 + ROWS].T
        pkm[:, 128:256] = x[b, n0:n0 + ROWS].T
        pkm[:, 256:768] = y[b].T
        pkm[:, 768:896] = Wa16
        pkm[:, 896:1024] = Wb16
        pkm[:, 1024:1152] = Wc16
        in_maps.append({"pk": np.ascontiguousarray(pkm), "cv": cvm})
    return in_maps


def kernel(x0, x, y, W1, b1, W2, b2):
    from concourse.bass_utils import run_bass_kernel_spmd

    nc = _build()
    in_maps = _prep_in_maps(x0, x, y, W1, b1, W2, b2)
    res = run_bass_kernel_spmd(nc, in_maps, list(range(NCORES)))
    kernel.last_result = res

    out = np.empty((B, N1, N2), np.float32)
    for ci in range(NCORES):
        o = res.results[ci]["outT"]  # [n within core, m]
        b = ci // (N1 // ROWS)
        n0 = (ci % (N1 // ROWS)) * ROWS
        out[b, n0:n0 + ROWS] = o
    return out


kernel.last_result = None



# revision 1
# speedup vs baseline: 14.0746x; 14.0746x over previous
"""Trainium2 Bass kernel for nn_CrossOutLayer_2 (dense pairwise MLP).

o[b,n,m] = sum_e W2[e]*gelu(hx[b,n,e] + hy[b,m,e] + b1[e]) + b2
  hx = x0 @ W1[:D] + x @ W1[D:2D],  hy = y @ W1[2D:]

Instead of evaluating gelu on all B*N1*N2*D elements (ACT-bound, ~66us),
approximate gelu with a separable harmonic expansion

  gelu(s) ~ g0 + 0.5 s + g2 s^2 + sum_{k=1..K} a_k cos(k w0 s)   (K=4: 1.3e-3)

Each term is separable in s = a + b (a = hx+b1 per n, b = hy per m) via the
angle-addition formula, so the whole pairwise map becomes 18 rank-128 fp16
matmuls on the PE:  o = sum_r F_r(a).T @ G_r(b):
  G basis {1, b, b^2, c1^j, s1*c1^(j-1)}: c1/s1 by ACT Sin (args <= 3.4 rad,
    inside the Sin spline's accurate range), monomials one fp16 DVE multiply
    each.
  F side: ACT Sin only at half-angle (args <= 2.9 rad), then fp16 Chebyshev
    recurrences on DVE for harmonics 2..K; per-partition scale/bias APs fold
    b1, phases, and w2; tensor_scalar applies the w2*coeff weights.

Sharded over (b, n1): each core owns 128 n-rows and all m.  Inputs packed
fp16 into one DMA; output written [n, m]-layout directly.
"""

import sys

sys.path.insert(0, "/opt/trn_rl_repo")

import numpy as np

B, N1, N2, D = 2, 512, 512, 128
NCORES = 8
ROWS = B * N1 // NCORES  # 128 n-rows per core
PKW = 128 * 5 + 512      # packed input width (x0T, xT, yT, Wa, Wb, Wc)
NCV = 25                 # const-vector columns

# fit constants (gelu ~ g0 + 0.5 s + g2 s^2 + sum_{k<=4} a_k cos(k w0 s))
FIT_G0 = 0.7486143130301098
FIT_W0 = 0.6532571942412266
FIT_A = (-0.5604016227440258, -0.14082111584080594,
         -0.03847842242402837, -0.007632327159040924)
FIT_G2 = 0.05209814114155775

_cache = {}


def _build(repeat=1, nmm=99, gchain=True, frec=True, weights=True,
           gacts=True, facts=True, dma_in=True, dma_out=True, pregemm=True,
           mmcombine=True, pipe_mode=True, unroll=2):
    key = ("nc", repeat, nmm, gchain, frec, weights, gacts, facts, dma_in,
           dma_out, pregemm, mmcombine, pipe_mode, unroll)
    if key in _cache:
        return _cache[key]
    import concourse.bacc as bacc
    import concourse.mybir as mybir
    import concourse.tile as tile

    f32 = mybir.dt.float32
    f16 = mybir.dt.float16
    SIN = mybir.ActivationFunctionType.Sin
    SQUARE = mybir.ActivationFunctionType.Square
    IDENT = mybir.ActivationFunctionType.Identity
    MULT = mybir.AluOpType.mult
    ADD = mybir.AluOpType.add
    w0 = FIT_W0

    nc = bacc.Bacc("TRN2", target_bir_lowering=False, debug=False)
    pk = nc.dram_tensor("pk", [D, PKW], f16, kind="ExternalInput")
    cv = nc.dram_tensor("cv", [D, NCV], f32, kind="ExternalInput")
    outT = nc.dram_tensor("outT", [D, N2], f32, kind="ExternalOutput")

    with tile.TileContext(nc) as tc:
        with (
            tc.tile_pool(name="const", bufs=1) as cpool,
            tc.tile_pool(name="work", bufs=2) as wpool,
            tc.tile_pool(name="psum", bufs=2, space="PSUM") as pspool,
        ):
            # ---- persistent constants (outside repeat loop) ----
            cv_sb = cpool.tile([D, NCV], f32, name="cv_sb", tag="cv_sb")
            nc.sync.dma_start(cv_sb[:], cv[:])
            ones_sb = cpool.tile([D, N2], f16, name="ones_sb", tag="ones_sb")
            nc.vector.memset(ones_sb[:], 1.0)
            fgq = cpool.tile([D, ROWS], f16, name="fgq", tag="fgq")
            nc.vector.tensor_scalar_mul(fgq[:], ones_sb[:, :ROWS], cv_sb[:, 10:11])

            def c(i):
                return cv_sb[:, i:i + 1]

            def load_stage(pipe, iv):
                pk_t = pipe.intermediate_tile([D, PKW], f16, name="pk_t")
                if dma_in:
                    nc.sync.dma_start(pk_t[:], pk[:])
                elif pregemm:
                    nc.vector.memset(pk_t[:], 0.01)
                return pk_t

            def compute_stage(pipe, iv, pk_sb):
                o_t = pipe.intermediate_tile([D, N2], f32, name="o_t")
                compute_body(pk_sb, o_t)
                return o_t

            def store_stage(pipe, iv, o_t):
                if dma_out:
                    nc.sync.dma_start(outT[:], o_t[:])

            def compute_body(pk_sb, o_sb):
                x0T = pk_sb[:, 0:128]
                xT = pk_sb[:, 128:256]
                yT = pk_sb[:, 256:768]
                Wa = pk_sb[:, 768:896]
                Wb = pk_sb[:, 896:1024]
                Wc = pk_sb[:, 1024:1152]

                hy_ps = pspool.tile([D, N2], f32, name="hy_ps", tag="hy")
                hx_ps = pspool.tile([D, N2], f32, name="hx_ps", tag="hx")
                if pregemm:
                    nc.tensor.matmul(hy_ps[:], Wc, yT, start=True, stop=True)
                    nc.tensor.matmul(hx_ps[:, :ROWS], Wa, x0T, start=True, stop=False)
                    nc.tensor.matmul(hx_ps[:, :ROWS], Wb, xT, start=False, stop=True)
                else:
                    nc.vector.memset(hy_ps[:], 0.1)
                    nc.vector.memset(hx_ps[:, :ROWS], 0.1)
                hx = hx_ps[:, :ROWS]

                def gtile(name):
                    return cpool.tile([D, N2], f16, name=name, tag=name)

                def ftile(name):
                    return cpool.tile([D, ROWS], f16, name=name, tag=name)

                # ---- ACT: G base first (gates the longest DVE chain) ----
                c1 = gtile("c1")
                s1 = gtile("s1")
                g1 = gtile("g1")
                gq = gtile("gq")
                if gacts:
                    nc.scalar.activation(c1[:], hy_ps[:], SIN, bias=c(7), scale=w0)
                    nc.scalar.activation(s1[:], hy_ps[:], SIN, bias=c(8), scale=w0)
                shA = ftile("shA")
                chA = ftile("chA")
                sqraw = ftile("sqraw")
                fm1 = ftile("fm1")
                fm5 = ftile("fm5")
                if facts:
                    nc.scalar.activation(shA[:], hx, SIN, bias=c(5), scale=w0 / 2)
                    nc.scalar.activation(chA[:], hx, SIN, bias=c(6), scale=w0 / 2)
                if gacts:
                    nc.scalar.activation(g1[:], hy_ps[:], IDENT, bias=c(8), scale=1.0)
                    nc.scalar.activation(gq[:], hy_ps[:], SQUARE, bias=c(8), scale=1.0)
                if facts:
                    nc.scalar.activation(sqraw[:], hx, SQUARE, bias=c(0), scale=1.0)
                    nc.scalar.activation(fm1[:], hx, IDENT, bias=c(2), scale=c(1))
                    nc.scalar.activation(fm5[:], hx, IDENT, bias=c(4), scale=c(3))
                if not gacts:
                    nc.vector.memset(c1[:], 0.3)
                    nc.vector.memset(s1[:], 0.3)
                    nc.vector.memset(g1[:], 0.3)
                    nc.vector.memset(gq[:], 0.3)
                if not facts:
                    nc.vector.memset(shA[:], 0.3)
                    nc.vector.memset(chA[:], 0.3)
                    nc.vector.memset(sqraw[:], 0.3)
                    nc.vector.memset(fm1[:], 0.3)
                    nc.vector.memset(fm5[:], 0.3)

                # ---- DVE: G monomial chain ----
                if gchain:
                    u2 = gtile("u2")
                    nc.vector.tensor_mul(u2[:], c1[:], c1[:])
                    v2 = gtile("v2")
                    nc.vector.tensor_mul(v2[:], s1[:], c1[:])
                    u3 = gtile("u3")
                    nc.vector.tensor_mul(u3[:], u2[:], c1[:])
                    v3 = gtile("v3")
                    nc.vector.tensor_mul(v3[:], v2[:], c1[:])
                    u4 = gtile("u4")
                    nc.vector.tensor_mul(u4[:], u3[:], c1[:])
                    v4 = gtile("v4")
                    nc.vector.tensor_mul(v4[:], v3[:], c1[:])
                else:
                    u2 = v2 = u3 = v3 = u4 = v4 = c1

                # ---- DVE: F-side fp16 Chebyshev recurrences ----
                if not frec:
                    c1r = ss = c2r = s2r = c3r = s3r = c4r = s4r = shA
                if frec:
                    sh2A = ftile("sh2A")
                    nc.vector.tensor_mul(sh2A[:], shA[:], shA[:])
                    c1r = ftile("c1r")  # cos(w0 a)
                    nc.vector.tensor_scalar(c1r[:], sh2A[:], -2.0, 1.0, MULT, ADD)
                    ss = ftile("ss")    # sin(w0 a)/2
                    nc.vector.tensor_mul(ss[:], shA[:], chA[:])
                    c1dd = ftile("c1dd")
                    nc.vector.tensor_scalar_mul(c1dd[:], c1r[:], 2.0)
                    c2t = ftile("c2t")
                    nc.vector.tensor_mul(c2t[:], c1dd[:], c1r[:])
                    c2r = ftile("c2r")
                    nc.vector.tensor_scalar_add(c2r[:], c2t[:], -1.0)
                    s2r = ftile("s2r")  # sin(2 w0 a)/2
                    nc.vector.tensor_mul(s2r[:], c1dd[:], ss[:])
                    c3t = ftile("c3t")
                    nc.vector.tensor_mul(c3t[:], c1dd[:], c2r[:])
                    c3r = ftile("c3r")
                    nc.vector.tensor_sub(c3r[:], c3t[:], c1r[:])
                    s3t = ftile("s3t")
                    nc.vector.tensor_mul(s3t[:], c1dd[:], s2r[:])
                    s3r = ftile("s3r")
                    nc.vector.tensor_sub(s3r[:], s3t[:], ss[:])
                    c4t = ftile("c4t")
                    nc.vector.tensor_mul(c4t[:], c1dd[:], c3r[:])
                    c4r = ftile("c4r")
                    nc.vector.tensor_sub(c4r[:], c4t[:], c2r[:])
                    s4t = ftile("s4t")
                    nc.vector.tensor_mul(s4t[:], c1dd[:], s3r[:])
                    s4r = ftile("s4r")
                    nc.vector.tensor_sub(s4r[:], s4t[:], s2r[:])

                # ---- DVE: per-partition weights ----
                def ts(name, raw, ci):
                    if not weights:
                        return raw
                    t = ftile(name)
                    nc.vector.tensor_scalar_mul(t[:], raw[:], c(ci))
                    return t

                def stt(name, raw, ci, addend):
                    t = ftile(name)
                    nc.vector.scalar_tensor_tensor(
                        t[:], raw[:], c(ci), addend[:], MULT, ADD
                    )
                    return t

                o_ps = pspool.tile([D, N2], f32, name="o_ps", tag="o_ps")
                if mmcombine and weights and frec:
                    # fold all terms sharing a G feature into one stationary
                    t1 = stt("t1", sqraw, 10, fm1)
                    t2 = stt("t2", c2r, 11, t1)
                    st_ones = stt("st_ones", c4r, 12, t2)
                    st_c1 = stt("st_c1", c3r, 14, ts("c1w", c1r, 13))
                    st_s1 = stt("st_s1", s3r, 20, ts("s1w", ss, 19))
                    st_u2 = stt("st_u2", c4r, 16, ts("c2wb", c2r, 15))
                    st_v2 = stt("st_v2", s4r, 22, ts("s2w", s2r, 21))
                    mms = [
                        (fm5, g1),
                        (fgq, gq),
                        (st_c1, c1),
                        (st_s1, s1),
                        (st_u2, u2),
                        (st_v2, v2),
                        (ts("c3wb", c3r, 17), u3),
                        (ts("s3wb", s3r, 23), v3),
                        (ts("c4wc", c4r, 18), u4),
                        (ts("s4wb", s4r, 24), v4),
                        (st_ones, ones_sb),
                    ]
                else:
                    sqw = ts("sqw", sqraw, 10)
                    c2wa = ts("c2wa", c2r, 11)
                    c4wa = ts("c4wa", c4r, 12)
                    c1w = ts("c1w", c1r, 13)
                    c3wa = ts("c3wa", c3r, 14)
                    c2wb = ts("c2wb", c2r, 15)
                    c4wb = ts("c4wb", c4r, 16)
                    c3wb = ts("c3wb", c3r, 17)
                    c4wc = ts("c4wc", c4r, 18)
                    s1w = ts("s1w", ss, 19)
                    s3wa = ts("s3wa", s3r, 20)
                    s2w = ts("s2w", s2r, 21)
                    s4wa = ts("s4wa", s4r, 22)
                    s3wb = ts("s3wb", s3r, 23)
                    s4wb = ts("s4wb", s4r, 24)
                    mms = [
                        (fm1, ones_sb),
                        (fm5, g1),
                        (c1w, c1),
                        (s1w, s1),
                        (sqw, ones_sb),
                        (fgq, gq),
                        (c2wa, ones_sb),
                        (c2wb, u2),
                        (s2w, v2),
                        (c3wa, c1),
                        (c3wb, u3),
                        (s3wa, s1),
                        (s3wb, v3),
                        (c4wa, ones_sb),
                        (c4wb, u2),
                        (c4wc, u4),
                        (s4wa, v2),
                        (s4wb, v4),
                    ]
                mms = mms[:nmm]
                for i, (F, G) in enumerate(mms):
                    nc.tensor.matmul(
                        o_ps[:], F[:], G[:],
                        start=(i == 0), stop=(i == len(mms) - 1),
                    )

                # ---- evac + b2 ----
                nc.scalar.activation(o_sb[:], o_ps[:], IDENT, bias=c(9), scale=1.0)

            def body():
                pk_sb = wpool.tile([D, PKW], f16, name="pk_sb", tag="pk_sb")
                if dma_in:
                    nc.sync.dma_start(pk_sb[:], pk[:])
                elif pregemm:
                    nc.vector.memset(pk_sb[:], 0.01)
                o_sb = cpool.tile([D, N2], f32, name="o_sb", tag="o_sb")
                compute_body(pk_sb, o_sb)
                if dma_out:
                    nc.sync.dma_start(outT[:], o_sb[:])

            if repeat == 1:
                body()
            elif pipe_mode:
                tc.For_i_pipelined(
                    [load_stage, compute_stage, store_stage], 0, repeat,
                    unroll=unroll, hint_engines=(mybir.EngineType.PE,),
                )
            else:
                with tc.For_i(0, repeat, 1, hint_engines=(mybir.EngineType.PE,)):
                    body()

    nc.compile()
    _cache[key] = nc
    return nc


def _prep_in_maps(x0, x, y, W1, b1, W2, b2):
    x0 = np.asarray(x0, np.float32)
    x = np.asarray(x, np.float32)
    y = np.asarray(y, np.float32)
    W1 = np.asarray(W1, np.float32)
    b1 = np.asarray(b1, np.float32)
    W2 = np.asarray(W2, np.float32)
    b2 = np.asarray(b2, np.float32)

    w2 = W2[:, 0]
    g0, w0, a, g2 = FIT_G0, FIT_W0, FIT_A, FIT_G2

    cvm = np.zeros((D, NCV), np.float32)
    cvm[:, 0] = b1
    cvm[:, 1] = 0.5 * w2
    cvm[:, 2] = w2 * (g0 + 0.5 * b1)
    cvm[:, 3] = 2.0 * g2 * w2
    cvm[:, 4] = w2 * (0.5 + 2.0 * g2 * b1)
    cvm[:, 5] = (w0 / 2) * b1
    cvm[:, 6] = (w0 / 2) * b1 + np.pi / 2
    cvm[:, 7] = np.pi / 2
    cvm[:, 8] = 0.0
    cvm[:, 9] = b2[0]
    cvm[:, 10] = w2 * g2
    cvm[:, 11] = -w2 * a[1]
    cvm[:, 12] = w2 * a[3]
    cvm[:, 13] = w2 * a[0]
    cvm[:, 14] = -3.0 * w2 * a[2]
    cvm[:, 15] = 2.0 * w2 * a[1]
    cvm[:, 16] = -8.0 * w2 * a[3]
    cvm[:, 17] = 4.0 * w2 * a[2]
    cvm[:, 18] = 8.0 * w2 * a[3]
    cvm[:, 19] = -2.0 * w2 * a[0]
    cvm[:, 20] = 2.0 * w2 * a[2]
    cvm[:, 21] = -4.0 * w2 * a[1]
    cvm[:, 22] = 8.0 * w2 * a[3]
    cvm[:, 23] = -8.0 * w2 * a[2]
    cvm[:, 24] = -16.0 * w2 * a[3]
    cvm = np.ascontiguousarray(cvm)

    Wa16 = W1[:D].astype(np.float16)
    Wb16 = W1[D:2 * D].astype(np.float16)
    Wc16 = W1[2 * D:].astype(np.float16)

    in_maps = []
    for ci in range(NCORES):
        b = ci // (N1 // ROWS)
        n0 = (ci % (N1 // ROWS)) * ROWS
        pkm = np.empty((D, PKW), np.float16)
        pkm[:, 0:128] = x0[b, n0:n0 + ROWS].T
        pkm[:, 128:256] = x[b, n0:n0 + ROWS].T
        pkm[:, 256:768] = y[b].T
        pkm[:, 768:896] = Wa16
        pkm[:, 896:1024] = Wb16
        pkm[:, 1024:1152] = Wc16
        in_maps.append({"pk": np.ascontiguousarray(pkm), "cv": cvm})
    return in_maps


def kernel(x0, x, y, W1, b1, W2, b2):
    from concourse.bass_utils import run_bass_kernel_spmd

    nc = _build()
    in_maps = _prep_in_maps(x0, x, y, W1, b1, W2, b2)
    res = run_bass_kernel_spmd(nc, in_maps, list(range(NCORES)))
    kernel.last_result = res

    out = np.empty((B, N1, N2), np.float32)
    for ci in range(NCORES):
        o = res.results[ci]["outT"]  # [n within core, m]
        b = ci // (N1 // ROWS)
        n0 = (ci % (N1 // ROWS)) * ROWS
        out[b, n0:n0 + ROWS] = o
    return out


kernel.last_result = None

